# revision 25
# baseline (speedup 1.0000x reference)
"""Multi-head attention (B=4, S=2048, H=1024, 16 heads) on 8 trn2 NeuronCores.

Sharding: data-parallel over batch (4) x tensor-parallel over head-groups (2):
core c handles batch c//2, heads 8*(c%2) .. 8*(c%2)+8. Each core computes its
partial output projection; host sums the two head-group partials + bo.

Per-core device algorithm (all matmul inputs bf16, fp32 accumulation):
  inputs are pre-transposed on host: xqT/xkT/xvT = x^T (hidden, seq)
  QT[t] (128=2 heads' d, sq)   = wqT-chunks^T @ xqT-chunks (+bias, per-partition)
  KT[t] likewise
  V[i]  (128 sk, 8*(64+1))     = xvT-chunks^T @ wvT (+bias), with a ones column
                                 appended per head (for softmax row sums)
  per head-pair t, sq-chunk s (512), sk-tile i (128):
     S^T = KT-slice^T @ QT-slice  (two heads row-packed in the PE array)
     P^T = exp(S^T * 0.125)       (ACT, psum->sbuf, bf16 out)
     ctx_aug (65, 512) += V_aug-slice^T @ P^T   (row 64 = softmax denominator)
  normalize: recip (DVE) -> broadcast via ones x recip matmul (PE, fp32) ->
     ctx^T = ctx * recip_bcast (DVE, bf16 out)
  out (sq, 1024) = sum_t ctx^T-chunks^T @ woT   -> DMA out (fp32)
"""
import os
import sys

sys.path.insert(0, "/opt/trn_rl_repo")

import numpy as np
import ml_dtypes

import concourse.bass as bass
import concourse.mybir as mybir
import concourse.tile as tile

# ---------------------------------------------------------------------------
# Walrus in this environment allows at most 1 sync wait per instruction (2 for
# EventSemaphore); Tile sometimes emits more (e.g. the exit drain). Hoist the
# extra waits onto EventSemaphore instructions inserted before the offender.
import json as _json


def _transform_bir_json(bir_bytes: bytes) -> bytes:
    bir = _json.loads(bir_bytes)
    changed = False
    ctr = 0
    for fn in bir.get("functions", []):
        for blk in fn.get("blocks", []):
            out = []
            for inst in blk.get("instructions", []):
                si = inst.get("sync_info") or {}
                waits = si.get("on_wait") or []
                cap = 2 if inst.get("opcode") == "EventSemaphore" else 1
                if len(waits) > cap:
                    changed = True
                    extra = waits[:-cap]
                    si["on_wait"] = waits[-cap:]
                    for i in range(0, len(extra), 2):
                        ctr += 1
                        out.append(
                            {
                                "debug": inst.get("debug"),
                                "engine": inst["engine"],
                                "ins": [],
                                "name": f"{inst['name']}_xw{ctr}",
                                "opcode": "EventSemaphore",
                                "outs": [],
                                "sync_info": {
                                    "on_update": [],
                                    "on_wait": extra[i : i + 2],
                                },
                            }
                        )
                out.append(inst)
            blk["instructions"] = out
    if not changed:
        return bir_bytes
    return _json.dumps(bir).encode()


def _apply_bir_patch():
    import concourse.bass_utils as bu
    import concourse.bass2jax as b2j

    if getattr(b2j, "_bir_waitfix_applied", False):
        return
    orig = bu.compile_bir_kernel

    def patched(bir_json, tmpdir, neff_name="file.neff"):
        return orig(_transform_bir_json(bir_json), tmpdir, neff_name)

    b2j.compile_bir_kernel = patched
    bu.compile_bir_kernel = patched
    b2j._bir_waitfix_applied = True


_apply_bir_patch()

from concourse.bass_utils import run_bass_kernel_spmd  # noqa: E402

# ---------------------------------------------------------------------------
HIDDEN = 1024
HEADS = 16
HD = 64  # head dim
B, SQ, SK = 4, 2048, 2048
NCORES = 8
HPC = 8  # heads per core (tensor-parallel over 2 head groups)
HL = HPC * HD  # local hidden slice = 512
SCALE = HD ** -0.5

F32 = mybir.dt.float32
BF16 = mybir.dt.bfloat16

_CACHED = {}


def _build_nc(dt_mm):
    nc = bass.Bass()
    xqT_d = nc.declare_dram_parameter("xqT", [HIDDEN, SQ], dt_mm, isOutput=False)
    xkT_d = nc.declare_dram_parameter("xkT", [HIDDEN, SK], dt_mm, isOutput=False)
    xvT_d = nc.declare_dram_parameter("xvT", [HIDDEN, SK], dt_mm, isOutput=False)
    wqT_d = nc.declare_dram_parameter("wqT", [HIDDEN, HL], dt_mm, isOutput=False)
    wkT_d = nc.declare_dram_parameter("wkT", [HIDDEN, HL], dt_mm, isOutput=False)
    wvT_d = nc.declare_dram_parameter("wvT", [HIDDEN, HL], dt_mm, isOutput=False)
    woT_d = nc.declare_dram_parameter("woT", [HL, HIDDEN], dt_mm, isOutput=False)
    bq_d = nc.declare_dram_parameter("bq2", [128, 4], F32, isOutput=False)
    bk_d = nc.declare_dram_parameter("bk2", [128, 4], F32, isOutput=False)
    bvb_d = nc.declare_dram_parameter("bvb", [128, HL], F32, isOutput=False)
    out_d = nc.declare_dram_parameter("out", [SQ, HIDDEN], F32, isOutput=True)

    NHC = HIDDEN // 128  # 8 hidden chunks
    NT = 4  # head-pair tiles (8 local heads -> 4 pairs of 64 rows)
    NS = 4  # sq chunks of 512
    NI = SK // 128  # 16 sk tiles

    with tile.TileContext(nc) as tc:
        from contextlib import ExitStack

        with ExitStack() as stack:
            wpool = stack.enter_context(tc.tile_pool(name="wpool", bufs=1))
            apool = stack.enter_context(tc.tile_pool(name="apool", bufs=1))

            # ---- persistent weights / biases (DMAs emitted at point of need)
            wq_sb = [wpool.tile([128, HL], dt_mm, name=f"wq{c}", tag=f"wq{c}") for c in range(NHC)]
            wk_sb = [wpool.tile([128, HL], dt_mm, name=f"wk{c}", tag=f"wk{c}") for c in range(NHC)]
            wv_sb = [wpool.tile([128, HL], dt_mm, name=f"wv{c}", tag=f"wv{c}") for c in range(NHC)]
            wo_sb = [wpool.tile([128, HIDDEN], dt_mm, name=f"wo{t}", tag=f"wo{t}") for t in range(NT)]
            bq_sb = wpool.tile([128, 4], F32)
            bk_sb = wpool.tile([128, 4], F32)
            bvb_sb = wpool.tile([128, HL], F32)

            # ---- persistent activations
            QT = [apool.tile([128, SQ], dt_mm, name=f"QT{t}", tag=f"QT{t}") for t in range(NT)]
            KT = [apool.tile([128, SK], dt_mm, name=f"KT{t}", tag=f"KT{t}") for t in range(NT)]
            # V[i]: per-head 128-col block [0:64]=V values, [64:128]=ones.
            # The ones half makes the ctx matmul emit the softmax denominator
            # replicated across psum rows 64..127 at zero extra PE cost
            # (matmul time depends only on N).
            V = [apool.tile([128, HPC * 128], dt_mm, name=f"V{i}", tag=f"V{i}") for i in range(NI)]
            CX = [apool.tile([128, SQ], dt_mm, name=f"CX{t}", tag=f"CX{t}") for t in range(NT)]

            inner = stack.enter_context(ExitStack())
            spool = inner.enter_context(tc.tile_pool(name="ldpool", bufs=2))
            dpool = inner.enter_context(tc.tile_pool(name="dpool", bufs=4))
            psA = inner.enter_context(tc.tile_pool(name="psA", bufs=2, space="PSUM"))
            psS = inner.enter_context(tc.tile_pool(name="psS", bufs=2, space="PSUM"))
            psC = inner.enter_context(tc.tile_pool(name="psC", bufs=1, space="PSUM"))

            def emit_proj(t, s, which):
                xT_d, w_sb, b_sb, OUT, nm = which
                xch = [
                    spool.tile([128, 512], dt_mm, name=f"x{nm}{t}{s}_{c}", tag=f"xch{c}")
                    for c in range(NHC)
                ]
                for c in range(NHC):
                    nc.sync.dma_start(
                        out=xch[c][:],
                        in_=xT_d[128 * c : 128 * c + 128, 512 * s : 512 * s + 512],
                    )
                ps = psA.tile([128, 512], F32, name=f"ps{nm}{s}{t}", tag="psA")
                for c in range(NHC):
                    nc.tensor.matmul(
                        ps[:],
                        w_sb[c][:, 128 * t : 128 * t + 128],
                        xch[c][:],
                        start=(c == 0),
                        stop=(c == NHC - 1),
                    )
                nc.vector.tensor_scalar_add(
                    OUT[t][:, 512 * s : 512 * s + 512], ps[:], b_sb[:, t : t + 1]
                )

            def emit_v_tile(i):
                ps = psA.tile([128, HL], F32, name=f"psv{i}", tag="psA")
                for c in range(NHC):
                    nc.tensor.matmul(
                        ps[:],
                        xv_sb[c][:, 128 * i : 128 * i + 128],
                        wv_sb[c][:],
                        start=(c == 0),
                        stop=(c == NHC - 1),
                    )
                nc.vector.memset(V[i][:], 1.0)
                vv = V[i].rearrange("p (h e) -> p h e", e=128)
                nc.vector.tensor_add(
                    vv[:, :, 0:HD],
                    ps[:].rearrange("p (h d) -> p h d", d=HD),
                    bvb_sb[:].rearrange("p (h d) -> p h d", d=HD),
                )

            def emit_outproj(q):
                # output projection for one finished q-tile; reuses the psA
                # slots that the (by now finished) projections vacated.
                ot = dpool.tile([128, HIDDEN], F32, name=f"ot{q}", tag="ot", bufs=2)
                for half in range(2):
                    po = psA.tile([128, 512], F32, name=f"po{q}_{half}", tag="psA")
                    for tt in range(NT):
                        nc.tensor.matmul(
                            po[:],
                            CX[tt][:, 128 * q : 128 * q + 128],
                            wo_sb[tt][:, 512 * half : 512 * half + 512],
                            start=(tt == 0),
                            stop=(tt == NT - 1),
                        )
                    nc.vector.tensor_copy(ot[:, 512 * half : 512 * half + 512], po[:])
                nc.sync.dma_start(out=out_d[128 * q : 128 * q + 128, :], in_=ot[:])

            def emit_attention_chunk(t, s, jit_v=False):
                sq = slice(512 * s, 512 * s + 512)
                ctx0 = psC.tile([128, 512], F32, name=f"c0_{t}{s}", tag="ctx0")
                ctx1 = psC.tile([128, 512], F32, name=f"c1_{t}{s}", tag="ctx1")
                for i in range(NI):
                    sk = slice(128 * i, 128 * i + 128)
                    st = psS.tile([128, 1024], F32, name=f"st{t}{s}{i}", tag="st")
                    nc.tensor.matmul(
                        st[:, 0:512],
                        KT[t][0:64, sk],
                        QT[t][0:64, sq],
                        start=True,
                        stop=True,
                        tile_position=(0, 0),
                    )
                    nc.tensor.matmul(
                        st[:, 512:1024],
                        KT[t][64:128, sk],
                        QT[t][64:128, sq],
                        start=True,
                        stop=True,
                        tile_position=(64, 0),
                    )
                    pt = dpool.tile([128, 1024], dt_mm, name=f"pt{t}{s}{i}", tag="pt", bufs=6)
                    nc.scalar.activation(
                        pt[:], st[:], mybir.ActivationFunctionType.Exp, scale=SCALE
                    )
                    if jit_v:
                        # produce V[i] just in time for its ctx matmul
                        emit_v_tile(i)
                    h0 = 2 * t
                    h1 = 2 * t + 1
                    nc.tensor.matmul(
                        ctx0[:],
                        V[i][:, 128 * h0 : 128 * h0 + 128],
                        pt[:, 0:512],
                        start=(i == 0),
                        stop=(i == NI - 1),
                    )
                    nc.tensor.matmul(
                        ctx1[:],
                        V[i][:, 128 * h1 : 128 * h1 + 128],
                        pt[:, 512:1024],
                        start=(i == 0),
                        stop=(i == NI - 1),
                    )
                    if t == NT - 1 and s > 0 and i >= 8 and i % 2 == 0:
                        # output projection for the previous (finished) s-chunk,
                        # placed late enough that its CX inputs (gated by the
                        # serial DVE epilogue) are ready when PE reaches it.
                        emit_outproj(4 * (s - 1) + (i - 8) // 2)
                # copy out of psum promptly (frees the single ctx bank), then
                # normalize from SBUF: rows 64:128 hold the replicated
                # softmax denominator.
                cxu0 = dpool.tile([128, 512], F32, name=f"u0_{t}{s}", tag="cxu0", bufs=1)
                cxu1 = dpool.tile([128, 512], F32, name=f"u1_{t}{s}", tag="cxu1", bufs=1)
                nc.vector.tensor_copy(cxu0[:], ctx0[:])
                nc.vector.tensor_copy(cxu1[:], ctx1[:])
                rb0 = dpool.tile([64, 512], F32, name=f"rb0_{t}{s}", tag="rb0", bufs=1)
                rb1 = dpool.tile([64, 512], F32, name=f"rb1_{t}{s}", tag="rb1", bufs=1)
                nc.vector.reciprocal(rb0[:], cxu0[64:128, :])
                nc.vector.reciprocal(rb1[:], cxu1[64:128, :])
                nc.vector.tensor_mul(CX[t][0:64, sq], cxu0[0:64, :], rb0[:])
                nc.vector.tensor_mul(CX[t][64:128, sq], cxu1[0:64, :], rb1[:])

            PROJ_Q = (xqT_d, wq_sb, bq_sb, QT, "q")
            PROJ_K = (xkT_d, wk_sb, bk_sb, KT, "k")

            # ---- t=0: QT/KT for s=0 first (the first exps need only these),
            # V projected just-in-time inside the first attention chunk.
            for c in range(NHC):
                nc.sync.dma_start(out=wq_sb[c][:], in_=wqT_d[128 * c : 128 * c + 128, :])
            nc.sync.dma_start(out=bq_sb[:], in_=bq_d[:])
            for s in range(NS):
                emit_proj(0, s, PROJ_Q)
            for c in range(NHC):
                nc.sync.dma_start(out=wk_sb[c][:], in_=wkT_d[128 * c : 128 * c + 128, :])
            nc.sync.dma_start(out=bk_sb[:], in_=bk_d[:])
            for s in range(NS):
                emit_proj(0, s, PROJ_K)
            for c in range(NHC):
                nc.sync.dma_start(out=wv_sb[c][:], in_=wvT_d[128 * c : 128 * c + 128, :])
            nc.sync.dma_start(out=bvb_sb[:], in_=bvb_d[:])
            xv_sb = [
                spool.tile([128, SK], dt_mm, name=f"xv{c}", tag=f"xf{c}", bufs=1) for c in range(NHC)
            ]
            for c in range(NHC):
                nc.sync.dma_start(out=xv_sb[c][:], in_=xvT_d[128 * c : 128 * c + 128, :])
            for t in range(NT):
                nc.sync.dma_start(out=wo_sb[t][:], in_=woT_d[128 * t : 128 * t + 128, :])
            for i in range(NI):
                emit_v_tile(i)
            for s in range(NS):
                emit_attention_chunk(0, s)

            # ---- t>=1: emit projections for t, then attention(t); the
            # scheduler overlaps them into attention's ACT-bound gaps.
            for t in range(1, NT):
                for s in range(NS):
                    emit_proj(t, s, PROJ_Q)
                    emit_proj(t, s, PROJ_K)
                for s in range(NS):
                    emit_attention_chunk(t, s)
            for q in range(4 * (NS - 1), 4 * NS):
                emit_outproj(q)

    return nc


def _get_nc():
    dt_mm = F32 if os.environ.get("MHA_FP32") == "1" else BF16
    key = str(dt_mm)
    if key not in _CACHED:
        _CACHED[key] = _build_nc(dt_mm)
    return _CACHED[key], dt_mm


def kernel(query, key, value, Wq, bq, Wk, bk, Wv, bv, Wo, bo):
    nc, dt_mm = _get_nc()
    np_mm = ml_dtypes.bfloat16 if dt_mm == BF16 else np.float32

    query = np.asarray(query, dtype=np.float32)
    key = np.asarray(key, dtype=np.float32)
    value = np.asarray(value, dtype=np.float32)
    Wq = np.asarray(Wq, dtype=np.float32)
    Wk = np.asarray(Wk, dtype=np.float32)
    Wv = np.asarray(Wv, dtype=np.float32)
    Wo = np.asarray(Wo, dtype=np.float32)
    bq = np.asarray(bq, dtype=np.float32)
    bk = np.asarray(bk, dtype=np.float32)
    bv = np.asarray(bv, dtype=np.float32)
    bo = np.asarray(bo, dtype=np.float32)

    in_maps = []
    for c in range(NCORES):
        b_idx, hg = c // 2, c % 2
        rows = slice(HL * hg, HL * hg + HL)
        in_maps.append(
            {
                "xqT": np.ascontiguousarray(query[b_idx].T).astype(np_mm),
                "xkT": np.ascontiguousarray(key[b_idx].T).astype(np_mm),
                "xvT": np.ascontiguousarray(value[b_idx].T).astype(np_mm),
                "wqT": np.ascontiguousarray(Wq[rows, :].T).astype(np_mm),
                "wkT": np.ascontiguousarray(Wk[rows, :].T).astype(np_mm),
                "wvT": np.ascontiguousarray(Wv[rows, :].T).astype(np_mm),
                "woT": np.ascontiguousarray(Wo[:, rows].T).astype(np_mm),
                "bq2": np.ascontiguousarray(bq[rows].reshape(4, 128).T),
                "bk2": np.ascontiguousarray(bk[rows].reshape(4, 128).T),
                "bvb": np.ascontiguousarray(np.broadcast_to(bv[rows], (128, HL))),
                "out": None,
            }
        )
    for m in in_maps:
        del m["out"]

    trace = os.environ.get("MHA_TRACE") == "1"
    res = run_bass_kernel_spmd(nc, in_maps, list(range(NCORES)), trace=trace)
    if trace:
        kernel.last_exec_time_ns = res.exec_time_ns
        kernel.last_results = res

    out = np.empty((B, SQ, HIDDEN), dtype=np.float32)
    for b_idx in range(B):
        out[b_idx] = res.results[2 * b_idx]["out"]
        out[b_idx] += res.results[2 * b_idx + 1]["out"]
    out += bo[None, None, :]
    return out


# revision 26
# speedup vs baseline: 1.0080x; 1.0080x over previous
"""Multi-head attention (B=4, S=2048, H=1024, 16 heads) on 8 trn2 NeuronCores.

Sharding: data-parallel over batch (4) x tensor-parallel over head-groups (2):
core c handles batch c//2, heads 8*(c%2) .. 8*(c%2)+8. Each core computes its
partial output projection; host sums the two head-group partials + bo.

Per-core device algorithm (all matmul inputs bf16, fp32 accumulation):
  inputs are pre-transposed on host: xqT/xkT/xvT = x^T (hidden, seq)
  QT[t] (128=2 heads' d, sq)   = wqT-chunks^T @ xqT-chunks (+bias, per-partition)
  KT[t] likewise
  V[i]  (128 sk, 8*(64+1))     = xvT-chunks^T @ wvT (+bias), with a ones column
                                 appended per head (for softmax row sums)
  per head-pair t, sq-chunk s (512), sk-tile i (128):
     S^T = KT-slice^T @ QT-slice  (two heads row-packed in the PE array)
     P^T = exp(S^T * 0.125)       (ACT, psum->sbuf, bf16 out)
     ctx_aug (65, 512) += V_aug-slice^T @ P^T   (row 64 = softmax denominator)
  normalize: recip (DVE) -> broadcast via ones x recip matmul (PE, fp32) ->
     ctx^T = ctx * recip_bcast (DVE, bf16 out)
  out (sq, 1024) = sum_t ctx^T-chunks^T @ woT   -> DMA out (fp32)
"""
import os
import sys

sys.path.insert(0, "/opt/trn_rl_repo")

import numpy as np
import ml_dtypes

import concourse.bass as bass
import concourse.mybir as mybir
import concourse.tile as tile

# ---------------------------------------------------------------------------
# Walrus in this environment allows at most 1 sync wait per instruction (2 for
# EventSemaphore); Tile sometimes emits more (e.g. the exit drain). Hoist the
# extra waits onto EventSemaphore instructions inserted before the offender.
import json as _json


def _transform_bir_json(bir_bytes: bytes) -> bytes:
    bir = _json.loads(bir_bytes)
    changed = False
    ctr = 0
    for fn in bir.get("functions", []):
        for blk in fn.get("blocks", []):
            out = []
            for inst in blk.get("instructions", []):
                si = inst.get("sync_info") or {}
                waits = si.get("on_wait") or []
                cap = 2 if inst.get("opcode") == "EventSemaphore" else 1
                if len(waits) > cap:
                    changed = True
                    extra = waits[:-cap]
                    si["on_wait"] = waits[-cap:]
                    for i in range(0, len(extra), 2):
                        ctr += 1
                        out.append(
                            {
                                "debug": inst.get("debug"),
                                "engine": inst["engine"],
                                "ins": [],
                                "name": f"{inst['name']}_xw{ctr}",
                                "opcode": "EventSemaphore",
                                "outs": [],
                                "sync_info": {
                                    "on_update": [],
                                    "on_wait": extra[i : i + 2],
                                },
                            }
                        )
                out.append(inst)
            blk["instructions"] = out
    if not changed:
        return bir_bytes
    return _json.dumps(bir).encode()


def _apply_bir_patch():
    import concourse.bass_utils as bu
    import concourse.bass2jax as b2j

    if getattr(b2j, "_bir_waitfix_applied", False):
        return
    orig = bu.compile_bir_kernel

    def patched(bir_json, tmpdir, neff_name="file.neff"):
        return orig(_transform_bir_json(bir_json), tmpdir, neff_name)

    b2j.compile_bir_kernel = patched
    bu.compile_bir_kernel = patched
    b2j._bir_waitfix_applied = True


_apply_bir_patch()

from concourse.bass_utils import run_bass_kernel_spmd  # noqa: E402

# ---------------------------------------------------------------------------
HIDDEN = 1024
HEADS = 16
HD = 64  # head dim
B, SQ, SK = 4, 2048, 2048
NCORES = 8
HPC = 8  # heads per core (tensor-parallel over 2 head groups)
HL = HPC * HD  # local hidden slice = 512
SCALE = HD ** -0.5

F32 = mybir.dt.float32
BF16 = mybir.dt.bfloat16

_CACHED = {}


def _build_nc(dt_mm):
    nc = bass.Bass()
    xqT_d = nc.declare_dram_parameter("xqT", [HIDDEN, SQ], dt_mm, isOutput=False)
    xkT_d = nc.declare_dram_parameter("xkT", [HIDDEN, SK], dt_mm, isOutput=False)
    xvT_d = nc.declare_dram_parameter("xvT", [HIDDEN, SK], dt_mm, isOutput=False)
    wqT_d = nc.declare_dram_parameter("wqT", [HIDDEN, HL], dt_mm, isOutput=False)
    wkT_d = nc.declare_dram_parameter("wkT", [HIDDEN, HL], dt_mm, isOutput=False)
    wvT_d = nc.declare_dram_parameter("wvT", [HIDDEN, HL], dt_mm, isOutput=False)
    woT_d = nc.declare_dram_parameter("woT", [HL, HIDDEN], dt_mm, isOutput=False)
    bq_d = nc.declare_dram_parameter("bq2", [128, 4], F32, isOutput=False)
    bk_d = nc.declare_dram_parameter("bk2", [128, 4], F32, isOutput=False)
    bvb_d = nc.declare_dram_parameter("bvb", [128, HL], F32, isOutput=False)
    out_d = nc.declare_dram_parameter("out", [SQ, HIDDEN], F32, isOutput=True)

    NHC = HIDDEN // 128  # 8 hidden chunks
    NT = 4  # head-pair tiles (8 local heads -> 4 pairs of 64 rows)
    NS = 4  # sq chunks of 512
    NI = SK // 128  # 16 sk tiles

    with tile.TileContext(nc) as tc:
        from contextlib import ExitStack

        with ExitStack() as stack:
            wpool = stack.enter_context(tc.tile_pool(name="wpool", bufs=1))
            apool = stack.enter_context(tc.tile_pool(name="apool", bufs=1))

            # ---- persistent weights / biases (DMAs emitted at point of need)
            wq_sb = [wpool.tile([128, HL], dt_mm, name=f"wq{c}", tag=f"wq{c}") for c in range(NHC)]
            wk_sb = [wpool.tile([128, HL], dt_mm, name=f"wk{c}", tag=f"wk{c}") for c in range(NHC)]
            wv_sb = [wpool.tile([128, HL], dt_mm, name=f"wv{c}", tag=f"wv{c}") for c in range(NHC)]
            wo_sb = [wpool.tile([128, HIDDEN], dt_mm, name=f"wo{t}", tag=f"wo{t}") for t in range(NT)]
            bq_sb = wpool.tile([128, 4], F32)
            bk_sb = wpool.tile([128, 4], F32)
            bvb_sb = wpool.tile([128, HL], F32)

            # ---- persistent activations
            QT = [apool.tile([128, SQ], dt_mm, name=f"QT{t}", tag=f"QT{t}") for t in range(NT)]
            # t=0's Q tiles are split per s-chunk so QT(0,s) projections can
            # be scheduled during attention(0,s') without a same-tile
            # write-during-read hazard.
            QT0s = [apool.tile([128, 512], dt_mm, name=f"QT0s{s}", tag=f"QT0s{s}") for s in range(NS)]
            KT = [apool.tile([128, SK], dt_mm, name=f"KT{t}", tag=f"KT{t}") for t in range(NT)]
            # V[i]: per-head 128-col block [0:64]=V values, [64:128]=ones.
            # The ones half makes the ctx matmul emit the softmax denominator
            # replicated across psum rows 64..127 at zero extra PE cost
            # (matmul time depends only on N).
            V = [apool.tile([128, HPC * 128], dt_mm, name=f"V{i}", tag=f"V{i}") for i in range(NI)]
            CX = [apool.tile([128, SQ], dt_mm, name=f"CX{t}", tag=f"CX{t}") for t in range(NT)]

            inner = stack.enter_context(ExitStack())
            spool = inner.enter_context(tc.tile_pool(name="ldpool", bufs=2))
            dpool = inner.enter_context(tc.tile_pool(name="dpool", bufs=4))
            psA = inner.enter_context(tc.tile_pool(name="psA", bufs=2, space="PSUM"))
            psS = inner.enter_context(tc.tile_pool(name="psS", bufs=2, space="PSUM"))
            psC = inner.enter_context(tc.tile_pool(name="psC", bufs=1, space="PSUM"))

            def emit_proj(t, s, which):
                xT_d, w_sb, b_sb, OUT, nm = which
                xch = [
                    spool.tile([128, 512], dt_mm, name=f"x{nm}{t}{s}_{c}", tag=f"xch{c}")
                    for c in range(NHC)
                ]
                for c in range(NHC):
                    nc.sync.dma_start(
                        out=xch[c][:],
                        in_=xT_d[128 * c : 128 * c + 128, 512 * s : 512 * s + 512],
                    )
                ps = psA.tile([128, 512], F32, name=f"ps{nm}{s}{t}", tag="psA")
                for c in range(NHC):
                    nc.tensor.matmul(
                        ps[:],
                        w_sb[c][:, 128 * t : 128 * t + 128],
                        xch[c][:],
                        start=(c == 0),
                        stop=(c == NHC - 1),
                    )
                if t == 0 and OUT is QT:
                    dst = QT0s[s][:, :]
                else:
                    dst = OUT[t][:, 512 * s : 512 * s + 512]
                nc.vector.tensor_scalar_add(dst, ps[:], b_sb[:, t : t + 1])

            def emit_v_tile(i):
                ps = psA.tile([128, HL], F32, name=f"psv{i}", tag="psA")
                for c in range(NHC):
                    nc.tensor.matmul(
                        ps[:],
                        xv_sb[c][:, 128 * i : 128 * i + 128],
                        wv_sb[c][:],
                        start=(c == 0),
                        stop=(c == NHC - 1),
                    )
                nc.vector.memset(V[i][:], 1.0)
                vv = V[i].rearrange("p (h e) -> p h e", e=128)
                nc.vector.tensor_add(
                    vv[:, :, 0:HD],
                    ps[:].rearrange("p (h d) -> p h d", d=HD),
                    bvb_sb[:].rearrange("p (h d) -> p h d", d=HD),
                )

            def emit_outproj(q):
                # output projection for one finished q-tile; reuses the psA
                # slots that the (by now finished) projections vacated.
                ot = dpool.tile([128, HIDDEN], F32, name=f"ot{q}", tag="ot", bufs=2)
                for half in range(2):
                    po = psA.tile([128, 512], F32, name=f"po{q}_{half}", tag="psA")
                    for tt in range(NT):
                        nc.tensor.matmul(
                            po[:],
                            CX[tt][:, 128 * q : 128 * q + 128],
                            wo_sb[tt][:, 512 * half : 512 * half + 512],
                            start=(tt == 0),
                            stop=(tt == NT - 1),
                        )
                    nc.vector.tensor_copy(ot[:, 512 * half : 512 * half + 512], po[:])
                nc.sync.dma_start(out=out_d[128 * q : 128 * q + 128, :], in_=ot[:])

            def emit_attention_chunk(t, s, jit_v=False):
                sq = slice(512 * s, 512 * s + 512)
                if t == 0:
                    qt_lo, qt_hi = QT0s[s][0:64, :], QT0s[s][64:128, :]
                else:
                    qt_lo, qt_hi = QT[t][0:64, sq], QT[t][64:128, sq]
                ctx0 = psC.tile([128, 512], F32, name=f"c0_{t}{s}", tag="ctx0")
                ctx1 = psC.tile([128, 512], F32, name=f"c1_{t}{s}", tag="ctx1")
                for i in range(NI):
                    sk = slice(128 * i, 128 * i + 128)
                    st = psS.tile([128, 1024], F32, name=f"st{t}{s}{i}", tag="st")
                    nc.tensor.matmul(
                        st[:, 0:512],
                        KT[t][0:64, sk],
                        qt_lo,
                        start=True,
                        stop=True,
                        tile_position=(0, 0),
                    )
                    nc.tensor.matmul(
                        st[:, 512:1024],
                        KT[t][64:128, sk],
                        qt_hi,
                        start=True,
                        stop=True,
                        tile_position=(64, 0),
                    )
                    pt = dpool.tile([128, 1024], dt_mm, name=f"pt{t}{s}{i}", tag="pt", bufs=6)
                    nc.scalar.activation(
                        pt[:], st[:], mybir.ActivationFunctionType.Exp, scale=SCALE
                    )
                    if jit_v:
                        # produce V[i] just in time for its ctx matmul
                        emit_v_tile(i)
                    h0 = 2 * t
                    h1 = 2 * t + 1
                    nc.tensor.matmul(
                        ctx0[:],
                        V[i][:, 128 * h0 : 128 * h0 + 128],
                        pt[:, 0:512],
                        start=(i == 0),
                        stop=(i == NI - 1),
                    )
                    nc.tensor.matmul(
                        ctx1[:],
                        V[i][:, 128 * h1 : 128 * h1 + 128],
                        pt[:, 512:1024],
                        start=(i == 0),
                        stop=(i == NI - 1),
                    )
                    if t == NT - 1 and s > 0 and i >= 12:
                        # output projection for the previous (finished) s-chunk,
                        # placed late enough that its CX inputs (gated by the
                        # serial DVE epilogue) are ready when PE reaches it.
                        emit_outproj(4 * (s - 1) + (i - 12))
                # copy out of psum promptly (frees the single ctx bank), then
                # normalize from SBUF: rows 64:128 hold the replicated
                # softmax denominator.
                cxu0 = dpool.tile([128, 512], F32, name=f"u0_{t}{s}", tag="cxu0", bufs=1)
                cxu1 = dpool.tile([128, 512], F32, name=f"u1_{t}{s}", tag="cxu1", bufs=1)
                nc.vector.tensor_copy(cxu0[:], ctx0[:])
                nc.vector.tensor_copy(cxu1[:], ctx1[:])
                rb0 = dpool.tile([64, 512], F32, name=f"rb0_{t}{s}", tag="rb0", bufs=1)
                rb1 = dpool.tile([64, 512], F32, name=f"rb1_{t}{s}", tag="rb1", bufs=1)
                nc.vector.reciprocal(rb0[:], cxu0[64:128, :])
                nc.vector.reciprocal(rb1[:], cxu1[64:128, :])
                nc.vector.tensor_mul(CX[t][0:64, sq], cxu0[0:64, :], rb0[:])
                nc.vector.tensor_mul(CX[t][64:128, sq], cxu1[0:64, :], rb1[:])

            PROJ_Q = (xqT_d, wq_sb, bq_sb, QT, "q")
            PROJ_K = (xkT_d, wk_sb, bk_sb, KT, "k")

            # ---- t=0: QT/KT for s=0 first (the first exps need only these),
            # V projected just-in-time inside the first attention chunk.
            for c in range(NHC):
                nc.sync.dma_start(out=wq_sb[c][:], in_=wqT_d[128 * c : 128 * c + 128, :])
            nc.sync.dma_start(out=bq_sb[:], in_=bq_d[:])
            emit_proj(0, 0, PROJ_Q)
            for c in range(NHC):
                nc.sync.dma_start(out=wv_sb[c][:], in_=wvT_d[128 * c : 128 * c + 128, :])
            nc.sync.dma_start(out=bvb_sb[:], in_=bvb_d[:])
            xv_sb = [
                spool.tile([128, SK], dt_mm, name=f"xv{c}", tag=f"xf{c}", bufs=1) for c in range(NHC)
            ]
            for c in range(NHC):
                nc.sync.dma_start(out=xv_sb[c][:], in_=xvT_d[128 * c : 128 * c + 128, :])
            for c in range(NHC):
                nc.sync.dma_start(out=wk_sb[c][:], in_=wkT_d[128 * c : 128 * c + 128, :])
            nc.sync.dma_start(out=bk_sb[:], in_=bk_d[:])
            for s in range(NS):
                emit_proj(0, s, PROJ_K)
            for t in range(NT):
                nc.sync.dma_start(out=wo_sb[t][:], in_=woT_d[128 * t : 128 * t + 128, :])
            emit_attention_chunk(0, 0, jit_v=True)
            for s in range(1, NS):
                emit_proj(0, s, PROJ_Q)
                emit_attention_chunk(0, s)

            # ---- t>=1: emit projections for t, then attention(t); the
            # scheduler overlaps them into attention's ACT-bound gaps.
            for t in range(1, NT):
                for s in range(NS):
                    emit_proj(t, s, PROJ_Q)
                    emit_proj(t, s, PROJ_K)
                for s in range(NS):
                    emit_attention_chunk(t, s)
            for q in range(4 * (NS - 1), 4 * NS):
                emit_outproj(q)

    return nc


def _get_nc():
    dt_mm = F32 if os.environ.get("MHA_FP32") == "1" else BF16
    key = str(dt_mm)
    if key not in _CACHED:
        _CACHED[key] = _build_nc(dt_mm)
    return _CACHED[key], dt_mm


def kernel(query, key, value, Wq, bq, Wk, bk, Wv, bv, Wo, bo):
    nc, dt_mm = _get_nc()
    np_mm = ml_dtypes.bfloat16 if dt_mm == BF16 else np.float32

    query = np.asarray(query, dtype=np.float32)
    key = np.asarray(key, dtype=np.float32)
    value = np.asarray(value, dtype=np.float32)
    Wq = np.asarray(Wq, dtype=np.float32)
    Wk = np.asarray(Wk, dtype=np.float32)
    Wv = np.asarray(Wv, dtype=np.float32)
    Wo = np.asarray(Wo, dtype=np.float32)
    bq = np.asarray(bq, dtype=np.float32)
    bk = np.asarray(bk, dtype=np.float32)
    bv = np.asarray(bv, dtype=np.float32)
    bo = np.asarray(bo, dtype=np.float32)

    in_maps = []
    for c in range(NCORES):
        b_idx, hg = c // 2, c % 2
        rows = slice(HL * hg, HL * hg + HL)
        in_maps.append(
            {
                "xqT": np.ascontiguousarray(query[b_idx].T).astype(np_mm),
                "xkT": np.ascontiguousarray(key[b_idx].T).astype(np_mm),
                "xvT": np.ascontiguousarray(value[b_idx].T).astype(np_mm),
                "wqT": np.ascontiguousarray(Wq[rows, :].T).astype(np_mm),
                "wkT": np.ascontiguousarray(Wk[rows, :].T).astype(np_mm),
                "wvT": np.ascontiguousarray(Wv[rows, :].T).astype(np_mm),
                "woT": np.ascontiguousarray(Wo[:, rows].T).astype(np_mm),
                "bq2": np.ascontiguousarray(bq[rows].reshape(4, 128).T),
                "bk2": np.ascontiguousarray(bk[rows].reshape(4, 128).T),
                "bvb": np.ascontiguousarray(np.broadcast_to(bv[rows], (128, HL))),
                "out": None,
            }
        )
    for m in in_maps:
        del m["out"]

    trace = os.environ.get("MHA_TRACE") == "1"
    res = run_bass_kernel_spmd(nc, in_maps, list(range(NCORES)), trace=trace)
    if trace:
        kernel.last_exec_time_ns = res.exec_time_ns
        kernel.last_results = res

    out = np.empty((B, SQ, HIDDEN), dtype=np.float32)
    for b_idx in range(B):
        out[b_idx] = res.results[2 * b_idx]["out"]
        out[b_idx] += res.results[2 * b_idx + 1]["out"]
    out += bo[None, None, :]
    return out


# revision 27
# speedup vs baseline: 1.0132x; 1.0051x over previous
"""Multi-head attention (B=4, S=2048, H=1024, 16 heads) on 8 trn2 NeuronCores.

Sharding: data-parallel over batch (4) x tensor-parallel over head-groups (2):
core c handles batch c//2, heads 8*(c%2) .. 8*(c%2)+8. Each core computes its
partial output projection; host sums the two head-group partials + bo.

Per-core device algorithm (all matmul inputs bf16, fp32 accumulation):
  inputs are pre-transposed on host: xqT/xkT/xvT = x^T (hidden, seq)
  QT[t] (128=2 heads' d, sq)   = wqT-chunks^T @ xqT-chunks (+bias, per-partition)
  KT[t] likewise
  V[i]  (128 sk, 8*(64+1))     = xvT-chunks^T @ wvT (+bias), with a ones column
                                 appended per head (for softmax row sums)
  per head-pair t, sq-chunk s (512), sk-tile i (128):
     S^T = KT-slice^T @ QT-slice  (two heads row-packed in the PE array)
     P^T = exp(S^T * 0.125)       (ACT, psum->sbuf, bf16 out)
     ctx_aug (65, 512) += V_aug-slice^T @ P^T   (row 64 = softmax denominator)
  normalize: recip (DVE) -> broadcast via ones x recip matmul (PE, fp32) ->
     ctx^T = ctx * recip_bcast (DVE, bf16 out)
  out (sq, 1024) = sum_t ctx^T-chunks^T @ woT   -> DMA out (fp32)
"""
import os
import sys

sys.path.insert(0, "/opt/trn_rl_repo")

import numpy as np
import ml_dtypes

import concourse.bass as bass
import concourse.mybir as mybir
import concourse.tile as tile

# ---------------------------------------------------------------------------
# Walrus in this environment allows at most 1 sync wait per instruction (2 for
# EventSemaphore); Tile sometimes emits more (e.g. the exit drain). Hoist the
# extra waits onto EventSemaphore instructions inserted before the offender.
import json as _json


def _transform_bir_json(bir_bytes: bytes) -> bytes:
    bir = _json.loads(bir_bytes)
    changed = False
    ctr = 0
    for fn in bir.get("functions", []):
        for blk in fn.get("blocks", []):
            out = []
            for inst in blk.get("instructions", []):
                si = inst.get("sync_info") or {}
                waits = si.get("on_wait") or []
                cap = 2 if inst.get("opcode") == "EventSemaphore" else 1
                if len(waits) > cap:
                    changed = True
                    extra = waits[:-cap]
                    si["on_wait"] = waits[-cap:]
                    for i in range(0, len(extra), 2):
                        ctr += 1
                        out.append(
                            {
                                "debug": inst.get("debug"),
                                "engine": inst["engine"],
                                "ins": [],
                                "name": f"{inst['name']}_xw{ctr}",
                                "opcode": "EventSemaphore",
                                "outs": [],
                                "sync_info": {
                                    "on_update": [],
                                    "on_wait": extra[i : i + 2],
                                },
                            }
                        )
                out.append(inst)
            blk["instructions"] = out
    if not changed:
        return bir_bytes
    return _json.dumps(bir).encode()


def _apply_bir_patch():
    import concourse.bass_utils as bu
    import concourse.bass2jax as b2j

    if getattr(b2j, "_bir_waitfix_applied", False):
        return
    orig = bu.compile_bir_kernel

    def patched(bir_json, tmpdir, neff_name="file.neff"):
        return orig(_transform_bir_json(bir_json), tmpdir, neff_name)

    b2j.compile_bir_kernel = patched
    bu.compile_bir_kernel = patched
    b2j._bir_waitfix_applied = True


_apply_bir_patch()

from concourse.bass_utils import run_bass_kernel_spmd  # noqa: E402

# ---------------------------------------------------------------------------
HIDDEN = 1024
HEADS = 16
HD = 64  # head dim
B, SQ, SK = 4, 2048, 2048
NCORES = 8
HPC = 8  # heads per core (tensor-parallel over 2 head groups)
HL = HPC * HD  # local hidden slice = 512
SCALE = HD ** -0.5

F32 = mybir.dt.float32
BF16 = mybir.dt.bfloat16

_CACHED = {}


def _build_nc(dt_mm):
    nc = bass.Bass()
    xqT_d = nc.declare_dram_parameter("xqT", [HIDDEN, SQ], dt_mm, isOutput=False)
    xkT_d = nc.declare_dram_parameter("xkT", [HIDDEN, SK], dt_mm, isOutput=False)
    xvT_d = nc.declare_dram_parameter("xvT", [HIDDEN, SK], dt_mm, isOutput=False)
    wqT_d = nc.declare_dram_parameter("wqT", [HIDDEN, HL], dt_mm, isOutput=False)
    wkT_d = nc.declare_dram_parameter("wkT", [HIDDEN, HL], dt_mm, isOutput=False)
    wvT_d = nc.declare_dram_parameter("wvT", [HIDDEN, HL], dt_mm, isOutput=False)
    woT_d = nc.declare_dram_parameter("woT", [HL, HIDDEN], dt_mm, isOutput=False)
    bq_d = nc.declare_dram_parameter("bq2", [128, 4], F32, isOutput=False)
    bk_d = nc.declare_dram_parameter("bk2", [128, 4], F32, isOutput=False)
    bvb_d = nc.declare_dram_parameter("bvb", [128, HL], F32, isOutput=False)
    out_d = nc.declare_dram_parameter("out", [SQ, HIDDEN], F32, isOutput=True)

    NHC = HIDDEN // 128  # 8 hidden chunks
    NT = 4  # head-pair tiles (8 local heads -> 4 pairs of 64 rows)
    NS = 4  # sq chunks of 512
    NI = SK // 128  # 16 sk tiles

    with tile.TileContext(nc) as tc:
        from contextlib import ExitStack

        with ExitStack() as stack:
            wpool = stack.enter_context(tc.tile_pool(name="wpool", bufs=1))
            apool = stack.enter_context(tc.tile_pool(name="apool", bufs=1))

            # ---- persistent weights / biases (DMAs emitted at point of need)
            wq_sb = [wpool.tile([128, HL], dt_mm, name=f"wq{c}", tag=f"wq{c}") for c in range(NHC)]
            wk_sb = [wpool.tile([128, HL], dt_mm, name=f"wk{c}", tag=f"wk{c}") for c in range(NHC)]
            wv_sb = [wpool.tile([128, HL], dt_mm, name=f"wv{c}", tag=f"wv{c}") for c in range(NHC)]
            wo_sb = [wpool.tile([128, HIDDEN], dt_mm, name=f"wo{t}", tag=f"wo{t}") for t in range(NT)]
            bq_sb = wpool.tile([128, 4], F32)
            bk_sb = wpool.tile([128, 4], F32)
            bvb_sb = wpool.tile([128, HL], F32)

            # ---- persistent activations
            QT = [apool.tile([128, SQ], dt_mm, name=f"QT{t}", tag=f"QT{t}") for t in range(NT)]
            # t=0's Q tiles are split per s-chunk so QT(0,s) projections can
            # be scheduled during attention(0,s') without a same-tile
            # write-during-read hazard.
            QT0s = [apool.tile([128, 512], dt_mm, name=f"QT0s{s}", tag=f"QT0s{s}") for s in range(NS)]
            KT = [apool.tile([128, SK], dt_mm, name=f"KT{t}", tag=f"KT{t}") for t in range(NT)]
            # V[i]: per-head 128-col block [0:64]=V values, [64:128]=ones.
            # The ones half makes the ctx matmul emit the softmax denominator
            # replicated across psum rows 64..127 at zero extra PE cost
            # (matmul time depends only on N).
            V = [apool.tile([128, HPC * 128], dt_mm, name=f"V{i}", tag=f"V{i}") for i in range(NI)]
            CX = [apool.tile([128, SQ], dt_mm, name=f"CX{t}", tag=f"CX{t}") for t in range(NT)]

            inner = stack.enter_context(ExitStack())
            spool = inner.enter_context(tc.tile_pool(name="ldpool", bufs=2))
            dpool = inner.enter_context(tc.tile_pool(name="dpool", bufs=4))
            psA = inner.enter_context(tc.tile_pool(name="psA", bufs=2, space="PSUM"))
            psS = inner.enter_context(tc.tile_pool(name="psS", bufs=2, space="PSUM"))
            psC = inner.enter_context(tc.tile_pool(name="psC", bufs=1, space="PSUM"))

            def emit_proj(t, s, which):
                xT_d, w_sb, b_sb, OUT, nm = which
                xch = [
                    spool.tile([128, 512], dt_mm, name=f"x{nm}{t}{s}_{c}", tag=f"xch{c}")
                    for c in range(NHC)
                ]
                for c in range(NHC):
                    nc.sync.dma_start(
                        out=xch[c][:],
                        in_=xT_d[128 * c : 128 * c + 128, 512 * s : 512 * s + 512],
                    )
                ps = psA.tile([128, 512], F32, name=f"ps{nm}{s}{t}", tag="psA")
                for c in range(NHC):
                    nc.tensor.matmul(
                        ps[:],
                        w_sb[c][:, 128 * t : 128 * t + 128],
                        xch[c][:],
                        start=(c == 0),
                        stop=(c == NHC - 1),
                    )
                if t == 0 and OUT is QT:
                    dst = QT0s[s][:, :]
                else:
                    dst = OUT[t][:, 512 * s : 512 * s + 512]
                nc.vector.tensor_scalar_add(dst, ps[:], b_sb[:, t : t + 1])

            def emit_v_tile(i):
                ps = psA.tile([128, HL], F32, name=f"psv{i}", tag="psA")
                for c in range(NHC):
                    nc.tensor.matmul(
                        ps[:],
                        xv_sb[c][:, 128 * i : 128 * i + 128],
                        wv_sb[c][:],
                        start=(c == 0),
                        stop=(c == NHC - 1),
                    )
                nc.vector.memset(V[i][:], 1.0)
                vv = V[i].rearrange("p (h e) -> p h e", e=128)
                nc.vector.tensor_add(
                    vv[:, :, 0:HD],
                    ps[:].rearrange("p (h d) -> p h d", d=HD),
                    bvb_sb[:].rearrange("p (h d) -> p h d", d=HD),
                )

            def emit_outproj(q):
                # output projection for one finished q-tile; reuses the psA
                # slots that the (by now finished) projections vacated.
                ot = dpool.tile([128, HIDDEN], F32, name=f"ot{q}", tag="ot", bufs=2)
                for half in range(2):
                    po = psA.tile([128, 512], F32, name=f"po{q}_{half}", tag="psA")
                    for tt in range(NT):
                        nc.tensor.matmul(
                            po[:],
                            CX[tt][:, 128 * q : 128 * q + 128],
                            wo_sb[tt][:, 512 * half : 512 * half + 512],
                            start=(tt == 0),
                            stop=(tt == NT - 1),
                        )
                    nc.vector.tensor_copy(ot[:, 512 * half : 512 * half + 512], po[:])
                nc.sync.dma_start(out=out_d[128 * q : 128 * q + 128, :], in_=ot[:])

            def emit_attention_chunk(t, s, jit_v=False):
                sq = slice(512 * s, 512 * s + 512)
                if t == 0:
                    qt_lo, qt_hi = QT0s[s][0:64, :], QT0s[s][64:128, :]
                else:
                    qt_lo, qt_hi = QT[t][0:64, sq], QT[t][64:128, sq]
                ctx0 = psC.tile([128, 512], F32, name=f"c0_{t}{s}", tag="ctx0")
                ctx1 = psC.tile([128, 512], F32, name=f"c1_{t}{s}", tag="ctx1")
                for i in range(NI):
                    sk = slice(128 * i, 128 * i + 128)
                    st = psS.tile([128, 1024], F32, name=f"st{t}{s}{i}", tag="st")
                    nc.tensor.matmul(
                        st[:, 0:512],
                        KT[t][0:64, sk],
                        qt_lo,
                        start=True,
                        stop=True,
                        tile_position=(0, 0),
                    )
                    nc.tensor.matmul(
                        st[:, 512:1024],
                        KT[t][64:128, sk],
                        qt_hi,
                        start=True,
                        stop=True,
                        tile_position=(64, 0),
                    )
                    pt = dpool.tile([128, 1024], dt_mm, name=f"pt{t}{s}{i}", tag="pt", bufs=6)
                    nc.scalar.activation(
                        pt[:], st[:], mybir.ActivationFunctionType.Exp, scale=SCALE
                    )
                    if jit_v:
                        # produce V[i] just in time for its ctx matmul
                        emit_v_tile(i)
                    h0 = 2 * t
                    h1 = 2 * t + 1
                    nc.tensor.matmul(
                        ctx0[:],
                        V[i][:, 128 * h0 : 128 * h0 + 128],
                        pt[:, 0:512],
                        start=(i == 0),
                        stop=(i == NI - 1),
                    )
                    nc.tensor.matmul(
                        ctx1[:],
                        V[i][:, 128 * h1 : 128 * h1 + 128],
                        pt[:, 512:1024],
                        start=(i == 0),
                        stop=(i == NI - 1),
                    )
                    if t == NT - 1 and s >= 2 and i >= 12:
                        # output projection for a finished s-chunk two
                        # iterations back, so its CX inputs (gated by the
                        # serial DVE epilogue) are ready wherever the
                        # scheduler places these matmuls.
                        emit_outproj(4 * (s - 2) + (i - 12))
                # copy out of psum promptly (frees the single ctx bank), then
                # normalize from SBUF: rows 64:128 hold the replicated
                # softmax denominator.
                cxu0 = dpool.tile([128, 512], F32, name=f"u0_{t}{s}", tag="cxu0", bufs=1)
                cxu1 = dpool.tile([128, 512], F32, name=f"u1_{t}{s}", tag="cxu1", bufs=1)
                nc.vector.tensor_copy(cxu0[:], ctx0[:])
                nc.vector.tensor_copy(cxu1[:], ctx1[:])
                rb0 = dpool.tile([64, 512], F32, name=f"rb0_{t}{s}", tag="rb0", bufs=1)
                rb1 = dpool.tile([64, 512], F32, name=f"rb1_{t}{s}", tag="rb1", bufs=1)
                nc.vector.reciprocal(rb0[:], cxu0[64:128, :])
                nc.vector.reciprocal(rb1[:], cxu1[64:128, :])
                nc.vector.tensor_mul(CX[t][0:64, sq], cxu0[0:64, :], rb0[:])
                nc.vector.tensor_mul(CX[t][64:128, sq], cxu1[0:64, :], rb1[:])

            PROJ_Q = (xqT_d, wq_sb, bq_sb, QT, "q")
            PROJ_K = (xkT_d, wk_sb, bk_sb, KT, "k")

            # ---- t=0: QT/KT for s=0 first (the first exps need only these),
            # V projected just-in-time inside the first attention chunk.
            for c in range(NHC):
                nc.sync.dma_start(out=wq_sb[c][:], in_=wqT_d[128 * c : 128 * c + 128, :])
            nc.sync.dma_start(out=bq_sb[:], in_=bq_d[:])
            emit_proj(0, 0, PROJ_Q)
            for c in range(NHC):
                nc.sync.dma_start(out=wv_sb[c][:], in_=wvT_d[128 * c : 128 * c + 128, :])
            nc.sync.dma_start(out=bvb_sb[:], in_=bvb_d[:])
            xv_sb = [
                spool.tile([128, SK], dt_mm, name=f"xv{c}", tag=f"xf{c}", bufs=1) for c in range(NHC)
            ]
            for c in range(NHC):
                nc.sync.dma_start(out=xv_sb[c][:], in_=xvT_d[128 * c : 128 * c + 128, :])
            for c in range(NHC):
                nc.sync.dma_start(out=wk_sb[c][:], in_=wkT_d[128 * c : 128 * c + 128, :])
            nc.sync.dma_start(out=bk_sb[:], in_=bk_d[:])
            for s in range(NS):
                emit_proj(0, s, PROJ_K)
            for t in range(NT):
                nc.sync.dma_start(out=wo_sb[t][:], in_=woT_d[128 * t : 128 * t + 128, :])
            emit_attention_chunk(0, 0, jit_v=True)
            for s in range(1, NS):
                emit_proj(0, s, PROJ_Q)
                emit_attention_chunk(0, s)

            # ---- t>=1: emit projections for t, then attention(t); the
            # scheduler overlaps them into attention's ACT-bound gaps.
            for t in range(1, NT):
                for s in range(NS):
                    emit_proj(t, s, PROJ_Q)
                    emit_proj(t, s, PROJ_K)
                for s in range(NS):
                    emit_attention_chunk(t, s)
            for q in range(4 * (NS - 2), 4 * NS):
                emit_outproj(q)

    return nc


def _get_nc():
    dt_mm = F32 if os.environ.get("MHA_FP32") == "1" else BF16
    key = str(dt_mm)
    if key not in _CACHED:
        _CACHED[key] = _build_nc(dt_mm)
    return _CACHED[key], dt_mm


def kernel(query, key, value, Wq, bq, Wk, bk, Wv, bv, Wo, bo):
    nc, dt_mm = _get_nc()
    np_mm = ml_dtypes.bfloat16 if dt_mm == BF16 else np.float32

    query = np.asarray(query, dtype=np.float32)
    key = np.asarray(key, dtype=np.float32)
    value = np.asarray(value, dtype=np.float32)
    Wq = np.asarray(Wq, dtype=np.float32)
    Wk = np.asarray(Wk, dtype=np.float32)
    Wv = np.asarray(Wv, dtype=np.float32)
    Wo = np.asarray(Wo, dtype=np.float32)
    bq = np.asarray(bq, dtype=np.float32)
    bk = np.asarray(bk, dtype=np.float32)
    bv = np.asarray(bv, dtype=np.float32)
    bo = np.asarray(bo, dtype=np.float32)

    in_maps = []
    for c in range(NCORES):
        b_idx, hg = c // 2, c % 2
        rows = slice(HL * hg, HL * hg + HL)
        in_maps.append(
            {
                "xqT": np.ascontiguousarray(query[b_idx].T).astype(np_mm),
                "xkT": np.ascontiguousarray(key[b_idx].T).astype(np_mm),
                "xvT": np.ascontiguousarray(value[b_idx].T).astype(np_mm),
                "wqT": np.ascontiguousarray(Wq[rows, :].T).astype(np_mm),
                "wkT": np.ascontiguousarray(Wk[rows, :].T).astype(np_mm),
                "wvT": np.ascontiguousarray(Wv[rows, :].T).astype(np_mm),
                "woT": np.ascontiguousarray(Wo[:, rows].T).astype(np_mm),
                "bq2": np.ascontiguousarray(bq[rows].reshape(4, 128).T),
                "bk2": np.ascontiguousarray(bk[rows].reshape(4, 128).T),
                "bvb": np.ascontiguousarray(np.broadcast_to(bv[rows], (128, HL))),
                "out": None,
            }
        )
    for m in in_maps:
        del m["out"]

    trace = os.environ.get("MHA_TRACE") == "1"
    res = run_bass_kernel_spmd(nc, in_maps, list(range(NCORES)), trace=trace)
    if trace:
        kernel.last_exec_time_ns = res.exec_time_ns
        kernel.last_results = res

    out = np.empty((B, SQ, HIDDEN), dtype=np.float32)
    for b_idx in range(B):
        out[b_idx] = res.results[2 * b_idx]["out"]
        out[b_idx] += res.results[2 * b_idx + 1]["out"]
    out += bo[None, None, :]
    return out
